# revision 21
# baseline (speedup 1.0000x reference)
"""Trainium2 Bass kernel for MultiHeadSelfAttention with RoPE.

Problem: x[2, 2048, 1024] @ W_qkv[1024, 3072] -> rope(q,k) -> softmax(q k^T/8) v
         -> out @ W_out[1024, 1024].

Sharding (8 cores): batch (2-way) x head-group (4-way, 4 heads each).
Each core computes a partial output [2048, 1024] = attnout_heads @ W_out_rows;
host sums the 4 head-group partials per batch.

v3 design:
 - bf16 everywhere except rope math and PSUM accumulation: x, W_qkv, W_out
   are host-cast to bf16 (halves DMA), q/k/v/exp/att_o tiles are bf16
   (same PE rate as f32r, half SBUF, 2x DVE).
 - Scores use K=64 matmuls on packed q/k tiles (measured same throughput
   as K=128; no zero-padded q).
 - Projection is fully in a chunk-paced preamble: x arrives in 512-col
   chunks and k/v/q projections for both pairs chase the DMA.
 - Attention is a pair-loop (2 heads/iter: 4 score MMs + 2 exps + 4
   attnv MMs). ACT does 2294ns/iter of exp vs PE's ~1730ns -> ScalarE
   stays saturated, which is the phase's roofline.
 - Softmax denominator rides as a ones-column in the attnv lhsT (row 64
   of the psum accumulator); normalization = GpSimd partition_broadcast
   + DVE reciprocal + multiply (no PSUM, no PE).
 - Output projection overlaps the tail of attention by rotating through
   the score PSUM slots (tags sA/sB), then drains.
"""

import sys

if "/opt/trn_rl_repo" not in sys.path:
    sys.path.insert(0, "/opt/trn_rl_repo")

import numpy as np

B, S, E = 2, 2048, 1024
ATT = 1024
H = 16
D = 64
HG = 4            # head groups (cores per batch)
HPG = H // HG     # heads per core = 4
PAIRS = HPG // 2  # head pairs per core = 2
ROPE_THETA = 10000.0
N_CORES = 8

EK = E // 128     # 8 contraction tiles over embedding dim
NCH = S // 512    # 4 x/proj column chunks
N_SK = S // 128   # 16 sk tiles
SQ_CHUNK = 1024   # q chunk for attention blocks
N_CH = S // SQ_CHUNK  # 2 attention chunks

_BUILT = {}


def _build_program():
    import concourse.bacc as bacc
    import concourse.tile as tile
    import concourse.mybir as mybir

    f32 = mybir.dt.float32
    f32r = mybir.dt.float32r
    bf16 = mybir.dt.bfloat16
    AF = mybir.ActivationFunctionType

    nc = bacc.Bacc(
        "TRN2",
        target_bir_lowering=False,
        debug=False,
        enable_asserts=False,
        num_devices=N_CORES,
    )

    xT = nc.dram_tensor("xT", [E, S], bf16, kind="ExternalInput").ap()
    w_qk = nc.dram_tensor("w_qk", [E, 2 * HPG * D], bf16, kind="ExternalInput").ap()
    w_v = nc.dram_tensor("w_v", [E, HPG * D], bf16, kind="ExternalInput").ap()
    w_o = nc.dram_tensor("w_o", [HPG * D, E], bf16, kind="ExternalInput").ap()
    cos_t = nc.dram_tensor("cos_t", [128, S], f32, kind="ExternalInput").ap()
    sin_t = nc.dram_tensor("sin_t", [128, S], f32, kind="ExternalInput").ap()
    mswap = nc.dram_tensor("mswap", [128, 128], f32r, kind="ExternalInput").ap()
    out = nc.dram_tensor("out", [S, E], bf16, kind="ExternalOutput").ap()

    with tile.TileContext(nc) as tc:
        with (
            tc.tile_pool(name="const", bufs=1) as constp,
            tc.tile_pool(name="persist", bufs=1) as pers,
            tc.tile_pool(name="xt", bufs=1) as xtp,
            tc.tile_pool(name="wqk", bufs=1) as wqkp,
            tc.tile_pool(name="wv", bufs=1) as wvp,
            tc.tile_pool(name="trig", bufs=1) as trigp,
            tc.tile_pool(name="exp", bufs=3) as expp,
            tc.tile_pool(name="oa", bufs=2) as oap,
            tc.tile_pool(name="ev", bufs=3) as evp,
        ):
            msw = constp.tile([128, 128], f32r, tag="msw")
            # (f32r/bf16 memsets are invalid ISA; memset f32 and copy.)
            ones_f32 = constp.tile([128, 64], f32, tag="ones_f32")
            nc.gpsimd.memset(ones_f32[:], 1.0)
            # prefetch the exp table-set (~2.7us ACT_TABLE_LOAD) so the
            # first real exp at the attention boundary doesn't stall the
            # pipeline (and HAM-cool the PE)
            warm = constp.tile([1, 64], f32, tag="warm")
            nc.scalar.activation(warm[:], ones_f32[0:1, :], AF.Exp, scale=1.0)

            # persistent attention tensors (all bf16)
            kT = [pers.tile([128, S], bf16, tag=f"kT{g}", name=f"kT{g}") for g in range(PAIRS)]
            qp = [pers.tile([128, S], bf16, tag=f"qp{g}", name=f"qp{g}") for g in range(PAIRS)]
            # v + ones-aug col per (pair g, head hh): cols 130g+65hh .. +65
            v_c = pers.tile([128, N_SK, 4 * 65], bf16, tag="vc")
            for hcol in range(4):
                nc.vector.tensor_copy(
                    v_c[:, :, 65 * hcol + 64], ones_f32[:, 0:N_SK]
                )
            att_o = [pers.tile([128, S], bf16, tag=f"ao{g}", name=f"ao{g}") for g in range(PAIRS)]
            wo_sb = [pers.tile([128, E], bf16, tag=f"wo{g}", name=f"wo{g}") for g in range(PAIRS)]

            cos_sb = trigp.tile([128, S], f32, tag="cos")
            sin_sb = trigp.tile([128, S], f32, tag="sin")

            # ---------------- DMA (ordered; x chunked so compute starts early)
            wqk_sb = []
            for e in range(EK):
                t = wqkp.tile([128, 2 * HPG * D], bf16, tag=f"wqk{e}", name=f"wqk{e}")
                nc.sync.dma_start(t[:], w_qk[128 * e : 128 * (e + 1), :])
                wqk_sb.append(t)
            xt_sb = [[None] * NCH for _ in range(EK)]
            for c in range(NCH):
                csl = slice(512 * c, 512 * (c + 1))
                for e in range(EK):
                    t = xtp.tile([128, 512], bf16, tag=f"xt{e}_{c}", name=f"xt{e}_{c}")
                    nc.sync.dma_start(t[:], xT[128 * e : 128 * (e + 1), csl])
                    xt_sb[e][c] = t
                nc.sync.dma_start(cos_sb[:, csl], cos_t[:, csl])
                nc.sync.dma_start(sin_sb[:, csl], sin_t[:, csl])
                if c == 0:
                    nc.sync.dma_start(msw[:], mswap[:])
                    wv_sb = []
                    for e in range(EK):
                        tv = wvp.tile([128, HPG * D], bf16, tag=f"wv{e}", name=f"wv{e}")
                        nc.sync.dma_start(tv[:], w_v[128 * e : 128 * (e + 1), :])
                        wv_sb.append(tv)
                if c == 1:
                    for g in range(PAIRS):
                        nc.sync.dma_start(wo_sb[g][:], w_o[128 * g : 128 * (g + 1), :])

            # ============ preamble: full projection, chunk-paced ============
            with (
                tc.tile_pool(name="raw", bufs=3) as rawp,
                tc.tile_pool(name="tt", bufs=4) as ttp,
                tc.tile_pool(name="pj", bufs=2, space="PSUM") as pjp,
                tc.tile_pool(name="rt", bufs=2, space="PSUM") as rtp,
                tc.tile_pool(name="vp", bufs=2, space="PSUM") as vpp,
            ):
                rope_pend = []

                def rope_tail():
                    if not rope_pend:
                        return
                    (g_, ti, sl, raw) = rope_pend.pop(0)
                    rp = rtp.tile([128, 512], f32, tag="rt",
                                  name=f"rot{g_}_{ti}_{sl.start}")
                    nc.tensor.matmul(rp[:], msw[:], raw[:], start=True, stop=True)
                    t2 = ttp.tile([128, 512], f32, tag="tt")
                    nc.vector.tensor_mul(t2[:], raw[:], cos_sb[:, sl])
                    t1 = ttp.tile([128, 512], f32, tag="tt")
                    nc.vector.tensor_mul(t1[:], rp[:], sin_sb[:, sl])
                    if ti == 1:
                        nc.vector.tensor_add(kT[g_][:, sl], t1[:], t2[:])
                    else:
                        nc.gpsimd.tensor_tensor(
                            qp[g_][:, sl], t1[:], t2[:], mybir.AluOpType.add
                        )

                def qk_chunk(g, ti, c):
                    coff = ti * HPG * D + 128 * g
                    sl = slice(512 * c, 512 * (c + 1))
                    pp = pjp.tile([128, 512], f32, tag="pj", name=f"pj{g}_{ti}_{c}")
                    for e in range(EK):
                        nc.tensor.matmul(
                            pp[:],
                            wqk_sb[e][:, coff : coff + 128],
                            xt_sb[e][c][:],
                            start=(e == 0),
                            stop=(e == EK - 1),
                        )
                    raw = rawp.tile([128, 512], f32r, tag="raw")
                    nc.vector.tensor_copy(raw[:], pp[:])
                    rope_pend.append((g, ti, sl, raw))
                    if len(rope_pend) > 1:
                        rope_tail()

                def v_st(st):
                    vp_ps = vpp.tile([128, 256], f32, tag="vp", name=f"vps{st}")
                    c, sub = divmod(st, 4)
                    for e in range(EK):
                        nc.tensor.matmul(
                            vp_ps[:],
                            xt_sb[e][c][:, 128 * sub : 128 * (sub + 1)],
                            wv_sb[e][:],
                            start=(e == 0),
                            stop=(e == EK - 1),
                        )
                    for h in range(4):
                        nc.vector.tensor_copy(
                            v_c[:, st, 65 * h : 65 * h + 64],
                            vp_ps[:, 64 * h : 64 * h + 64],
                        )

                for c in range(NCH):
                    qk_chunk(0, 1, c)            # k pair 0
                    qk_chunk(1, 1, c)            # k pair 1
                    qk_chunk(0, 0, c)            # q pair 0
                    qk_chunk(1, 0, c)            # q pair 1
                    # v last: its DVE copies have no downstream PE
                    # dependency, so the rope chains (which gate the first
                    # scores) never queue behind them
                    for st in range(4 * c, 4 * c + 4):
                        v_st(st)
                rope_tail()
                rope_tail()

            # ============ attention: pair-loop, ScalarE-saturated ===========
            fill = []

            with (
                tc.tile_pool(name="sAB", bufs=1, space="PSUM") as sABp,
                tc.tile_pool(name="oT", bufs=1, space="PSUM") as oTp,
            ):

                def out_items(st, n, tag):
                    ssl = slice(128 * st, 128 * (st + 1))
                    nsl = slice(512 * n, 512 * (n + 1))
                    cell = {}

                    def mk_mm(g):
                        def it():
                            if g == 0:
                                cell["op"] = sABp.tile(
                                    [128, 512], f32, tag=tag,
                                    name=f"op{st}_{n}"
                                )
                            nc.tensor.matmul(
                                cell["op"][:],
                                att_o[g][:, ssl],
                                wo_sb[g][:, nsl],
                                start=(g == 0),
                                stop=(g == PAIRS - 1),
                            )
                        return it

                    def ev_it():
                        ev = evp.tile([128, 512], bf16, tag="ev")
                        nc.vector.tensor_copy(ev[:], cell["op"][:])
                        nc.sync.dma_start(out[ssl, nsl], ev[:])

                    return [mk_mm(g) for g in range(PAIRS)] + [ev_it]

                def norm_head(g, ch, hh, oT):
                    # one fast 65-partition copy frees the PSUM accumulator
                    # for the next block; the slow single-partition staging
                    # and broadcast run asynchronously off SBUF afterwards.
                    rows = slice(64 * hh, 64 * hh + 64)
                    csl = slice(SQ_CHUNK * ch, SQ_CHUNK * (ch + 1))
                    oA = oap.tile([65, SQ_CHUNK], f32, tag="oA")
                    nc.vector.tensor_copy(oA[:], oT[:])
                    dn = oap.tile([1, SQ_CHUNK], f32, tag="dn")
                    nc.vector.tensor_copy(dn[:], oA[64:65, :])
                    rbb = oap.tile([64, SQ_CHUNK], f32, tag="rbb")
                    nc.gpsimd.partition_broadcast(rbb[:], dn[:])
                    rbr = oap.tile([64, SQ_CHUNK], f32, tag="rbr")
                    nc.vector.reciprocal_approx_fast(rbr[:], rbb[:])
                    nc.vector.tensor_mul(att_o[g][rows, csl], oA[0:64, :], rbr[:])

                def attention_block(g, ch):
                    oTA = oTp.tile([65, SQ_CHUNK], f32, tag="oTA",
                                   name=f"oTA{g}_{ch}")
                    oTB = oTp.tile([65, SQ_CHUNK], f32, tag="oTB",
                                   name=f"oTB{g}_{ch}")
                    exps = []

                    def attnv(j):
                        eA, eB = exps[j]
                        first, last = j == 0, j == N_SK - 1
                        for n in range(2):
                            nsl = slice(512 * n, 512 * (n + 1))
                            nc.tensor.matmul(
                                oTA[:, nsl], v_c[:, j, 130 * g : 130 * g + 65],
                                eA[:, nsl], start=first, stop=last,
                            )
                            nc.tensor.matmul(
                                oTB[:, nsl],
                                v_c[:, j, 130 * g + 65 : 130 * g + 130],
                                eB[:, nsl], start=first, stop=last,
                            )

                    for sk in range(N_SK):
                        sksl = slice(128 * sk, 128 * (sk + 1))
                        sA = sABp.tile([128, SQ_CHUNK], f32, tag="sA",
                                       name=f"sA{g}_{ch}_{sk}")
                        sB = sABp.tile([128, SQ_CHUNK], f32, tag="sB",
                                       name=f"sB{g}_{ch}_{sk}")
                        for n in range(2):
                            gsl = slice(SQ_CHUNK * ch + 512 * n,
                                        SQ_CHUNK * ch + 512 * (n + 1))
                            nc.tensor.matmul(
                                sA[:, 512 * n : 512 * (n + 1)],
                                kT[g][0:64, sksl], qp[g][0:64, gsl],
                                start=True, stop=True,
                            )
                            nc.tensor.matmul(
                                sB[:, 512 * n : 512 * (n + 1)],
                                kT[g][64:128, sksl], qp[g][64:128, gsl],
                                start=True, stop=True,
                            )
                        eA = expp.tile([128, SQ_CHUNK], bf16, tag="eA")
                        nc.scalar.activation(eA[:], sA[:], AF.Exp, scale=0.125)
                        eB = expp.tile([128, SQ_CHUNK], bf16, tag="eB")
                        nc.scalar.activation(eB[:], sB[:], AF.Exp, scale=0.125)
                        exps.append((eA, eB))
                        if sk > 0:
                            attnv(sk - 1)
                        for _ in range(2):
                            if fill:
                                fill.pop(0)()
                    attnv(N_SK - 1)
                    norm_head(g, ch, 0, oTA)
                    norm_head(g, ch, 1, oTB)

                for ch in range(N_CH):
                    for g in range(PAIRS):
                        attention_block(g, ch)
                    for st in range(8 * ch, 8 * ch + 8):
                        for n in range(2):
                            fill.extend(
                                out_items(st, n, "sA" if (st + n) % 2 == 0 else "sB")
                            )

                while fill:
                    fill.pop(0)()

    nc.compile()
    return nc


def _get_program():
    if "nc" not in _BUILT:
        _BUILT["nc"] = _build_program()
    return _BUILT["nc"]


def _host_inputs(x, W_qkv, W_out):
    """Build the 8 per-core input maps."""
    import ml_dtypes

    f = np.float32
    bf = ml_dtypes.bfloat16
    x = np.asarray(x, dtype=f)
    W_qkv = np.asarray(W_qkv, dtype=f)
    W_out = np.asarray(W_out, dtype=f)

    inv_freq = 1.0 / (ROPE_THETA ** (np.arange(0, D, 2, dtype=np.float64) / D))
    p = np.arange(128)
    freq_row = inv_freq[(p % D) // 2]  # [128]
    ang = freq_row[:, None] * np.arange(S, dtype=np.float64)[None, :]  # [128, S]
    cos_t = np.cos(ang).astype(f)
    sign = np.where(p % 2 == 0, -1.0, 1.0)[:, None]
    sin_t = (np.sin(ang) * sign).astype(f)

    msw = np.zeros((128, 128), dtype=f)
    msw[p, p ^ 1] = 1.0

    maps = []
    for core in range(N_CORES):
        b, hg = divmod(core, HG)
        hs = [HPG * hg + i for i in range(HPG)]
        w_qk = np.concatenate(
            [W_qkv[:, h * D : (h + 1) * D] for h in hs]
            + [W_qkv[:, ATT + h * D : ATT + (h + 1) * D] for h in hs],
            axis=1,
        )
        w_v = np.concatenate(
            [W_qkv[:, 2 * ATT + h * D : 2 * ATT + (h + 1) * D] for h in hs], axis=1
        )
        w_o = np.concatenate([W_out[h * D : (h + 1) * D, :] for h in hs], axis=0)
        maps.append(
            {
                "xT": np.ascontiguousarray(x[b].T).astype(bf),
                "w_qk": np.ascontiguousarray(w_qk).astype(bf),
                "w_v": np.ascontiguousarray(w_v).astype(bf),
                "w_o": np.ascontiguousarray(w_o).astype(bf),
                "cos_t": cos_t,
                "sin_t": sin_t,
                "mswap": msw,
            }
        )
    return maps


def kernel(x, W_qkv, W_out):
    from concourse.bass_utils import run_bass_kernel_spmd

    nc = _get_program()
    maps = _host_inputs(x, W_qkv, W_out)
    res = run_bass_kernel_spmd(nc, maps, core_ids=list(range(N_CORES)))
    out = np.zeros((B, S, E), dtype=np.float32)
    for core in range(N_CORES):
        b = core // HG
        out[b] += np.asarray(res.results[core]["out"], dtype=np.float32)
    return out


# revision 22
# speedup vs baseline: 1.2133x; 1.2133x over previous
"""Trainium2 Bass kernel for MultiHeadSelfAttention with RoPE.

Problem: x[2, 2048, 1024] @ W_qkv[1024, 3072] -> rope(q,k) -> softmax(q k^T/8) v
         -> out @ W_out[1024, 1024].

Sharding (8 cores): batch (2-way) x head-group (4-way, 4 heads each).
Each core computes a partial output [2048, 1024] = attnout_heads @ W_out_rows;
host sums the 4 head-group partials per batch.

v3 design:
 - bf16 everywhere except rope math and PSUM accumulation: x, W_qkv, W_out
   are host-cast to bf16 (halves DMA), q/k/v/exp/att_o tiles are bf16
   (same PE rate as f32r, half SBUF, 2x DVE).
 - Scores use K=64 matmuls on packed q/k tiles (measured same throughput
   as K=128; no zero-padded q).
 - Projection is fully in a chunk-paced preamble: x arrives in 512-col
   chunks and k/v/q projections for both pairs chase the DMA.
 - Attention is a pair-loop (2 heads/iter: 4 score MMs + 2 exps + 4
   attnv MMs). ACT does 2294ns/iter of exp vs PE's ~1730ns -> ScalarE
   stays saturated, which is the phase's roofline.
 - Softmax denominator rides as a ones-column in the attnv lhsT (row 64
   of the psum accumulator); normalization = GpSimd partition_broadcast
   + DVE reciprocal + multiply (no PSUM, no PE).
 - Output projection overlaps the tail of attention by rotating through
   the score PSUM slots (tags sA/sB), then drains.
"""

import sys

if "/opt/trn_rl_repo" not in sys.path:
    sys.path.insert(0, "/opt/trn_rl_repo")

import numpy as np

B, S, E = 2, 2048, 1024
ATT = 1024
H = 16
D = 64
HG = 4            # head groups (cores per batch)
HPG = H // HG     # heads per core = 4
PAIRS = HPG // 2  # head pairs per core = 2
ROPE_THETA = 10000.0
N_CORES = 8

EK = E // 128     # 8 contraction tiles over embedding dim
NCH = S // 512    # 4 x/proj column chunks
N_SK = S // 128   # 16 sk tiles
SQ_CHUNK = 1024   # q chunk for attention blocks
N_CH = S // SQ_CHUNK  # 2 attention chunks

_BUILT = {}


def _build_program():
    import concourse.bacc as bacc
    import concourse.tile as tile
    import concourse.mybir as mybir

    f32 = mybir.dt.float32
    f32r = mybir.dt.float32r
    bf16 = mybir.dt.bfloat16
    AF = mybir.ActivationFunctionType

    nc = bacc.Bacc(
        "TRN2",
        target_bir_lowering=False,
        debug=False,
        enable_asserts=False,
        num_devices=N_CORES,
    )

    xT = nc.dram_tensor("xT", [E, S], bf16, kind="ExternalInput").ap()
    w_qk = nc.dram_tensor("w_qk", [E, 2 * HPG * D], bf16, kind="ExternalInput").ap()
    w_v = nc.dram_tensor("w_v", [E, HPG * D], bf16, kind="ExternalInput").ap()
    w_o = nc.dram_tensor("w_o", [HPG * D, E], bf16, kind="ExternalInput").ap()
    cos_t = nc.dram_tensor("cos_t", [128, S], f32, kind="ExternalInput").ap()
    sin_t = nc.dram_tensor("sin_t", [128, S], f32, kind="ExternalInput").ap()
    mswap = nc.dram_tensor("mswap", [128, 128], f32r, kind="ExternalInput").ap()
    out = nc.dram_tensor("out", [S, E], bf16, kind="ExternalOutput").ap()

    with tile.TileContext(nc) as tc:
        with (
            tc.tile_pool(name="const", bufs=1) as constp,
            tc.tile_pool(name="persist", bufs=1) as pers,
            tc.tile_pool(name="xt", bufs=1) as xtp,
            tc.tile_pool(name="wqk", bufs=1) as wqkp,
            tc.tile_pool(name="wv", bufs=1) as wvp,
            tc.tile_pool(name="trig", bufs=1) as trigp,
            tc.tile_pool(name="exp", bufs=3) as expp,
            tc.tile_pool(name="oa", bufs=2) as oap,
            tc.tile_pool(name="ev", bufs=3) as evp,
        ):
            msw = constp.tile([128, 128], f32r, tag="msw")
            # (f32r/bf16 memsets are invalid ISA; memset f32 and copy.)
            ones_f32 = constp.tile([128, 64], f32, tag="ones_f32")
            nc.gpsimd.memset(ones_f32[:], 1.0)
            # prefetch the exp table-set (~2.7us ACT_TABLE_LOAD) so the
            # first real exp at the attention boundary doesn't stall the
            # pipeline (and HAM-cool the PE)
            warm = constp.tile([1, 64], f32, tag="warm")
            nc.scalar.activation(warm[:], ones_f32[0:1, :], AF.Exp, scale=1.0)

            # persistent attention tensors (all bf16)
            kT = [pers.tile([128, S], bf16, tag=f"kT{g}", name=f"kT{g}") for g in range(PAIRS)]
            qp = [pers.tile([128, S], bf16, tag=f"qp{g}", name=f"qp{g}") for g in range(PAIRS)]
            # v + ones-aug col per (pair g, head hh): cols 130g+65hh .. +65
            v_c = pers.tile([128, N_SK, 4 * 65], bf16, tag="vc")
            for hcol in range(4):
                nc.vector.tensor_copy(
                    v_c[:, :, 65 * hcol + 64], ones_f32[:, 0:N_SK]
                )
            att_o = [pers.tile([128, S], bf16, tag=f"ao{g}", name=f"ao{g}") for g in range(PAIRS)]
            wo_sb = [pers.tile([128, E], bf16, tag=f"wo{g}", name=f"wo{g}") for g in range(PAIRS)]

            cos_sb = trigp.tile([128, S], f32, tag="cos")
            sin_sb = trigp.tile([128, S], f32, tag="sin")

            # ---------------- DMA (ordered; x chunked so compute starts early)
            wqk_sb = []
            for e in range(EK):
                t = wqkp.tile([128, 2 * HPG * D], bf16, tag=f"wqk{e}", name=f"wqk{e}")
                nc.sync.dma_start(t[:], w_qk[128 * e : 128 * (e + 1), :])
                wqk_sb.append(t)
            # full-row x tiles: 4KB contiguous DMA rows (2KB rows halve DMA
            # throughput); chunk-paced compute slices columns out of them
            xt_full = []
            for e in range(EK):
                t = xtp.tile([128, S], bf16, tag=f"xtf{e}", name=f"xtf{e}")
                xt_full.append(t)
            xt_sb = [
                [xt_full[e][:, 512 * c : 512 * (c + 1)] for c in range(NCH)]
                for e in range(EK)
            ]
            for c in range(NCH):
                csl = slice(512 * c, 512 * (c + 1))
                for e in range(EK):
                    nc.sync.dma_start(
                        xt_full[e][:, csl], xT[128 * e : 128 * (e + 1), csl]
                    )
                nc.sync.dma_start(cos_sb[:, csl], cos_t[:, csl])
                nc.sync.dma_start(sin_sb[:, csl], sin_t[:, csl])
                if c == 0:
                    nc.sync.dma_start(msw[:], mswap[:])
                    wv_sb = []
                    for e in range(EK):
                        tv = wvp.tile([128, HPG * D], bf16, tag=f"wv{e}", name=f"wv{e}")
                        nc.sync.dma_start(tv[:], w_v[128 * e : 128 * (e + 1), :])
                        wv_sb.append(tv)
                if c == 1:
                    for g in range(PAIRS):
                        nc.sync.dma_start(wo_sb[g][:], w_o[128 * g : 128 * (g + 1), :])

            # ============ preamble: full projection, chunk-paced ============
            with (
                tc.tile_pool(name="raw", bufs=3) as rawp,
                tc.tile_pool(name="tt", bufs=4) as ttp,
                tc.tile_pool(name="pj", bufs=2, space="PSUM") as pjp,
                tc.tile_pool(name="rt", bufs=2, space="PSUM") as rtp,
                tc.tile_pool(name="vp", bufs=2, space="PSUM") as vpp,
            ):
                rope_pend = []

                def rope_tail():
                    if not rope_pend:
                        return
                    (g_, ti, sl, raw) = rope_pend.pop(0)
                    rp = rtp.tile([128, 512], f32, tag="rt",
                                  name=f"rot{g_}_{ti}_{sl.start}")
                    nc.tensor.matmul(rp[:], msw[:], raw[:], start=True, stop=True)
                    t2 = ttp.tile([128, 512], f32, tag="tt")
                    nc.vector.tensor_mul(t2[:], raw[:], cos_sb[:, sl])
                    t1 = ttp.tile([128, 512], f32, tag="tt")
                    nc.vector.tensor_mul(t1[:], rp[:], sin_sb[:, sl])
                    if ti == 1:
                        nc.vector.tensor_add(kT[g_][:, sl], t1[:], t2[:])
                    else:
                        nc.gpsimd.tensor_tensor(
                            qp[g_][:, sl], t1[:], t2[:], mybir.AluOpType.add
                        )

                def qk_chunk(g, ti, c):
                    coff = ti * HPG * D + 128 * g
                    sl = slice(512 * c, 512 * (c + 1))
                    pp = pjp.tile([128, 512], f32, tag="pj", name=f"pj{g}_{ti}_{c}")
                    for e in range(EK):
                        nc.tensor.matmul(
                            pp[:],
                            wqk_sb[e][:, coff : coff + 128],
                            xt_sb[e][c],
                            start=(e == 0),
                            stop=(e == EK - 1),
                        )
                    raw = rawp.tile([128, 512], f32r, tag="raw")
                    nc.vector.tensor_copy(raw[:], pp[:])
                    rope_pend.append((g, ti, sl, raw))
                    if len(rope_pend) > 1:
                        rope_tail()

                def v_st(st):
                    vp_ps = vpp.tile([128, 256], f32, tag="vp", name=f"vps{st}")
                    c, sub = divmod(st, 4)
                    for e in range(EK):
                        nc.tensor.matmul(
                            vp_ps[:],
                            xt_sb[e][c][:, 128 * sub : 128 * (sub + 1)],
                            wv_sb[e][:],
                            start=(e == 0),
                            stop=(e == EK - 1),
                        )
                    for h in range(4):
                        nc.vector.tensor_copy(
                            v_c[:, st, 65 * h : 65 * h + 64],
                            vp_ps[:, 64 * h : 64 * h + 64],
                        )

                for c in range(NCH):
                    qk_chunk(0, 1, c)            # k pair 0
                    qk_chunk(1, 1, c)            # k pair 1
                    qk_chunk(0, 0, c)            # q pair 0
                    qk_chunk(1, 0, c)            # q pair 1
                    # v last: its DVE copies have no downstream PE
                    # dependency, so the rope chains (which gate the first
                    # scores) never queue behind them
                    for st in range(4 * c, 4 * c + 4):
                        v_st(st)
                rope_tail()
                rope_tail()

            # ============ attention: pair-loop, ScalarE-saturated ===========
            fill = []

            with (
                tc.tile_pool(name="sAB", bufs=1, space="PSUM") as sABp,
                tc.tile_pool(name="oT", bufs=1, space="PSUM") as oTp,
            ):

                def out_items(st, n, tag):
                    ssl = slice(128 * st, 128 * (st + 1))
                    nsl = slice(512 * n, 512 * (n + 1))
                    cell = {}

                    def mk_mm(g):
                        def it():
                            if g == 0:
                                cell["op"] = sABp.tile(
                                    [128, 512], f32, tag=tag,
                                    name=f"op{st}_{n}"
                                )
                            nc.tensor.matmul(
                                cell["op"][:],
                                att_o[g][:, ssl],
                                wo_sb[g][:, nsl],
                                start=(g == 0),
                                stop=(g == PAIRS - 1),
                            )
                        return it

                    def ev_it():
                        ev = evp.tile([128, 512], bf16, tag="ev")
                        nc.vector.tensor_copy(ev[:], cell["op"][:])
                        nc.sync.dma_start(out[ssl, nsl], ev[:])

                    return [mk_mm(g) for g in range(PAIRS)] + [ev_it]

                def norm_head(g, ch, hh, oT):
                    # one fast 65-partition copy frees the PSUM accumulator
                    # for the next block; the slow single-partition staging
                    # and broadcast run asynchronously off SBUF afterwards.
                    rows = slice(64 * hh, 64 * hh + 64)
                    csl = slice(SQ_CHUNK * ch, SQ_CHUNK * (ch + 1))
                    oA = oap.tile([65, SQ_CHUNK], f32, tag="oA")
                    nc.vector.tensor_copy(oA[:], oT[:])
                    dn = oap.tile([1, SQ_CHUNK], f32, tag="dn")
                    nc.vector.tensor_copy(dn[:], oA[64:65, :])
                    rbb = oap.tile([64, SQ_CHUNK], f32, tag="rbb")
                    nc.gpsimd.partition_broadcast(rbb[:], dn[:])
                    rbr = oap.tile([64, SQ_CHUNK], f32, tag="rbr")
                    nc.vector.reciprocal_approx_fast(rbr[:], rbb[:])
                    nc.vector.tensor_mul(att_o[g][rows, csl], oA[0:64, :], rbr[:])

                def attention_block(g, ch):
                    oTA = oTp.tile([65, SQ_CHUNK], f32, tag="oTA",
                                   name=f"oTA{g}_{ch}")
                    oTB = oTp.tile([65, SQ_CHUNK], f32, tag="oTB",
                                   name=f"oTB{g}_{ch}")
                    exps = []

                    def attnv(j):
                        eA, eB = exps[j]
                        first, last = j == 0, j == N_SK - 1
                        for n in range(2):
                            nsl = slice(512 * n, 512 * (n + 1))
                            nc.tensor.matmul(
                                oTA[:, nsl], v_c[:, j, 130 * g : 130 * g + 65],
                                eA[:, nsl], start=first, stop=last,
                            )
                            nc.tensor.matmul(
                                oTB[:, nsl],
                                v_c[:, j, 130 * g + 65 : 130 * g + 130],
                                eB[:, nsl], start=first, stop=last,
                            )

                    for sk in range(N_SK):
                        sksl = slice(128 * sk, 128 * (sk + 1))
                        sA = sABp.tile([128, SQ_CHUNK], f32, tag="sA",
                                       name=f"sA{g}_{ch}_{sk}")
                        sB = sABp.tile([128, SQ_CHUNK], f32, tag="sB",
                                       name=f"sB{g}_{ch}_{sk}")
                        for n in range(2):
                            gsl = slice(SQ_CHUNK * ch + 512 * n,
                                        SQ_CHUNK * ch + 512 * (n + 1))
                            nc.tensor.matmul(
                                sA[:, 512 * n : 512 * (n + 1)],
                                kT[g][0:64, sksl], qp[g][0:64, gsl],
                                start=True, stop=True,
                            )
                            nc.tensor.matmul(
                                sB[:, 512 * n : 512 * (n + 1)],
                                kT[g][64:128, sksl], qp[g][64:128, gsl],
                                start=True, stop=True,
                            )
                        eA = expp.tile([128, SQ_CHUNK], bf16, tag="eA")
                        nc.scalar.activation(eA[:], sA[:], AF.Exp, scale=0.125)
                        eB = expp.tile([128, SQ_CHUNK], bf16, tag="eB")
                        nc.scalar.activation(eB[:], sB[:], AF.Exp, scale=0.125)
                        exps.append((eA, eB))
                        if sk > 0:
                            attnv(sk - 1)
                        if fill:
                            fill.pop(0)()
                    attnv(N_SK - 1)
                    norm_head(g, ch, 0, oTA)
                    norm_head(g, ch, 1, oTB)

                for ch in range(N_CH):
                    for g in range(PAIRS):
                        attention_block(g, ch)
                    for st in range(8 * ch, 8 * ch + 8):
                        for n in range(2):
                            fill.extend(
                                out_items(st, n, "sA" if (st + n) % 2 == 0 else "sB")
                            )

                while fill:
                    fill.pop(0)()

    nc.compile()
    return nc


def _get_program():
    if "nc" not in _BUILT:
        _BUILT["nc"] = _build_program()
    return _BUILT["nc"]


def _host_inputs(x, W_qkv, W_out):
    """Build the 8 per-core input maps."""
    import ml_dtypes

    f = np.float32
    bf = ml_dtypes.bfloat16
    x = np.asarray(x, dtype=f)
    W_qkv = np.asarray(W_qkv, dtype=f)
    W_out = np.asarray(W_out, dtype=f)

    inv_freq = 1.0 / (ROPE_THETA ** (np.arange(0, D, 2, dtype=np.float64) / D))
    p = np.arange(128)
    freq_row = inv_freq[(p % D) // 2]  # [128]
    ang = freq_row[:, None] * np.arange(S, dtype=np.float64)[None, :]  # [128, S]
    cos_t = np.cos(ang).astype(f)
    sign = np.where(p % 2 == 0, -1.0, 1.0)[:, None]
    sin_t = (np.sin(ang) * sign).astype(f)

    msw = np.zeros((128, 128), dtype=f)
    msw[p, p ^ 1] = 1.0

    maps = []
    for core in range(N_CORES):
        b, hg = divmod(core, HG)
        hs = [HPG * hg + i for i in range(HPG)]
        w_qk = np.concatenate(
            [W_qkv[:, h * D : (h + 1) * D] for h in hs]
            + [W_qkv[:, ATT + h * D : ATT + (h + 1) * D] for h in hs],
            axis=1,
        )
        w_v = np.concatenate(
            [W_qkv[:, 2 * ATT + h * D : 2 * ATT + (h + 1) * D] for h in hs], axis=1
        )
        w_o = np.concatenate([W_out[h * D : (h + 1) * D, :] for h in hs], axis=0)
        maps.append(
            {
                "xT": np.ascontiguousarray(x[b].T).astype(bf),
                "w_qk": np.ascontiguousarray(w_qk).astype(bf),
                "w_v": np.ascontiguousarray(w_v).astype(bf),
                "w_o": np.ascontiguousarray(w_o).astype(bf),
                "cos_t": cos_t,
                "sin_t": sin_t,
                "mswap": msw,
            }
        )
    return maps


def kernel(x, W_qkv, W_out):
    from concourse.bass_utils import run_bass_kernel_spmd

    nc = _get_program()
    maps = _host_inputs(x, W_qkv, W_out)
    res = run_bass_kernel_spmd(nc, maps, core_ids=list(range(N_CORES)))
    out = np.zeros((B, S, E), dtype=np.float32)
    for core in range(N_CORES):
        b = core // HG
        out[b] += np.asarray(res.results[core]["out"], dtype=np.float32)
    return out


# revision 23
# speedup vs baseline: 1.3500x; 1.1127x over previous
"""Trainium2 Bass kernel for MultiHeadSelfAttention with RoPE.

Problem: x[2, 2048, 1024] @ W_qkv[1024, 3072] -> rope(q,k) -> softmax(q k^T/8) v
         -> out @ W_out[1024, 1024].

Sharding (8 cores): batch (2-way) x head-group (4-way, 4 heads each).
Each core computes a partial output [2048, 1024] = attnout_heads @ W_out_rows;
host sums the 4 head-group partials per batch.

v3 design:
 - bf16 everywhere except rope math and PSUM accumulation: x, W_qkv, W_out
   are host-cast to bf16 (halves DMA), q/k/v/exp/att_o tiles are bf16
   (same PE rate as f32r, half SBUF, 2x DVE).
 - Scores use K=64 matmuls on packed q/k tiles (measured same throughput
   as K=128; no zero-padded q).
 - Projection is fully in a chunk-paced preamble: x arrives in 512-col
   chunks and k/v/q projections for both pairs chase the DMA.
 - Attention is a pair-loop (2 heads/iter: 4 score MMs + 2 exps + 4
   attnv MMs). ACT does 2294ns/iter of exp vs PE's ~1730ns -> ScalarE
   stays saturated, which is the phase's roofline.
 - Softmax denominator rides as a ones-column in the attnv lhsT (row 64
   of the psum accumulator); normalization = GpSimd partition_broadcast
   + DVE reciprocal + multiply (no PSUM, no PE).
 - Output projection overlaps the tail of attention by rotating through
   the score PSUM slots (tags sA/sB), then drains.
"""

import sys

if "/opt/trn_rl_repo" not in sys.path:
    sys.path.insert(0, "/opt/trn_rl_repo")

import numpy as np

B, S, E = 2, 2048, 1024
ATT = 1024
H = 16
D = 64
HG = 4            # head groups (cores per batch)
HPG = H // HG     # heads per core = 4
PAIRS = HPG // 2  # head pairs per core = 2
ROPE_THETA = 10000.0
N_CORES = 8

EK = E // 128     # 8 contraction tiles over embedding dim
NCH = S // 512    # 4 x/proj column chunks
N_SK = S // 128   # 16 sk tiles
SQ_CHUNK = 1024   # q chunk for attention blocks
N_CH = S // SQ_CHUNK  # 2 attention chunks

_BUILT = {}


def _build_program():
    import concourse.bacc as bacc
    import concourse.tile as tile
    import concourse.mybir as mybir

    f32 = mybir.dt.float32
    f32r = mybir.dt.float32r
    bf16 = mybir.dt.bfloat16
    AF = mybir.ActivationFunctionType

    nc = bacc.Bacc(
        "TRN2",
        target_bir_lowering=False,
        debug=False,
        enable_asserts=False,
        num_devices=N_CORES,
    )

    xT = nc.dram_tensor("xT", [E, S], bf16, kind="ExternalInput").ap()
    w_qk = nc.dram_tensor("w_qk", [E, 2 * HPG * D], bf16, kind="ExternalInput").ap()
    w_v = nc.dram_tensor("w_v", [E, HPG * D], bf16, kind="ExternalInput").ap()
    w_o = nc.dram_tensor("w_o", [HPG * D, E], bf16, kind="ExternalInput").ap()
    cos_t = nc.dram_tensor("cos_t", [128, S], f32, kind="ExternalInput").ap()
    sin_t = nc.dram_tensor("sin_t", [128, S], f32, kind="ExternalInput").ap()
    mswap = nc.dram_tensor("mswap", [128, 128], f32r, kind="ExternalInput").ap()
    out = nc.dram_tensor("out", [S, E], bf16, kind="ExternalOutput").ap()

    with tile.TileContext(nc) as tc:
        with (
            tc.tile_pool(name="const", bufs=1) as constp,
            tc.tile_pool(name="persist", bufs=1) as pers,
            tc.tile_pool(name="xt", bufs=1) as xtp,
            tc.tile_pool(name="wqk", bufs=1) as wqkp,
            tc.tile_pool(name="wv", bufs=1) as wvp,
            tc.tile_pool(name="trig", bufs=1) as trigp,
            tc.tile_pool(name="exp", bufs=3) as expp,
            tc.tile_pool(name="oa", bufs=2) as oap,
            tc.tile_pool(name="ev", bufs=3) as evp,
        ):
            msw = constp.tile([128, 128], f32r, tag="msw")
            # (f32r/bf16 memsets are invalid ISA; memset f32 and copy.)
            ones_f32 = constp.tile([128, 64], f32, tag="ones_f32")
            nc.gpsimd.memset(ones_f32[:], 1.0)
            # prefetch the exp table-set (~2.7us ACT_TABLE_LOAD) so the
            # first real exp at the attention boundary doesn't stall the
            # pipeline (and HAM-cool the PE)
            warm = constp.tile([1, 64], f32, tag="warm")
            nc.scalar.activation(warm[:], ones_f32[0:1, :], AF.Exp, scale=1.0)

            # persistent attention tensors (all bf16)
            kT = [pers.tile([128, S], bf16, tag=f"kT{g}", name=f"kT{g}") for g in range(PAIRS)]
            qp = [pers.tile([128, S], bf16, tag=f"qp{g}", name=f"qp{g}") for g in range(PAIRS)]
            # v + ones-aug col per (pair g, head hh): cols 130g+65hh .. +65
            v_c = pers.tile([128, N_SK, 4 * 65], bf16, tag="vc")
            for hcol in range(4):
                nc.vector.tensor_copy(
                    v_c[:, :, 65 * hcol + 64], ones_f32[:, 0:N_SK]
                )
            att_o = [pers.tile([128, S], bf16, tag=f"ao{g}", name=f"ao{g}") for g in range(PAIRS)]
            wo_sb = [pers.tile([128, E], bf16, tag=f"wo{g}", name=f"wo{g}") for g in range(PAIRS)]

            cos_sb = trigp.tile([128, S], f32, tag="cos")
            sin_sb = trigp.tile([128, S], f32, tag="sin")

            # ---------------- DMA (ordered; x chunked so compute starts early)
            wqk_sb = []
            for e in range(EK):
                t = wqkp.tile([128, 2 * HPG * D], bf16, tag=f"wqk{e}", name=f"wqk{e}")
                nc.sync.dma_start(t[:], w_qk[128 * e : 128 * (e + 1), :])
                wqk_sb.append(t)
            # full-row x tiles: 4KB contiguous DMA rows (2KB rows halve DMA
            # throughput); chunk-paced compute slices columns out of them
            xt_full = []
            for e in range(EK):
                t = xtp.tile([128, S], bf16, tag=f"xtf{e}", name=f"xtf{e}")
                xt_full.append(t)
            xt_sb = [
                [xt_full[e][:, 512 * c : 512 * (c + 1)] for c in range(NCH)]
                for e in range(EK)
            ]
            for c in range(NCH):
                csl = slice(512 * c, 512 * (c + 1))
                for e in range(EK):
                    nc.sync.dma_start(
                        xt_full[e][:, csl], xT[128 * e : 128 * (e + 1), csl]
                    )
                nc.sync.dma_start(cos_sb[:, csl], cos_t[:, csl])
                nc.sync.dma_start(sin_sb[:, csl], sin_t[:, csl])
                if c == 0:
                    nc.sync.dma_start(msw[:], mswap[:])
                    wv_sb = []
                    for e in range(EK):
                        tv = wvp.tile([128, HPG * D], bf16, tag=f"wv{e}", name=f"wv{e}")
                        nc.sync.dma_start(tv[:], w_v[128 * e : 128 * (e + 1), :])
                        wv_sb.append(tv)
                if c == 1:
                    for g in range(PAIRS):
                        nc.sync.dma_start(wo_sb[g][:], w_o[128 * g : 128 * (g + 1), :])

            # ============ preamble: full projection, chunk-paced ============
            with (
                tc.tile_pool(name="raw", bufs=3) as rawp,
                tc.tile_pool(name="tt", bufs=4) as ttp,
                tc.tile_pool(name="pj", bufs=2, space="PSUM") as pjp,
                tc.tile_pool(name="rt", bufs=2, space="PSUM") as rtp,
                tc.tile_pool(name="vp", bufs=2, space="PSUM") as vpp,
            ):
                rope_pend = []

                def rope_tail():
                    if not rope_pend:
                        return
                    (g_, ti, sl, raw) = rope_pend.pop(0)
                    rp = rtp.tile([128, 512], f32, tag="rt",
                                  name=f"rot{g_}_{ti}_{sl.start}")
                    nc.tensor.matmul(rp[:], msw[:], raw[:], start=True, stop=True)
                    t2 = ttp.tile([128, 512], f32, tag="tt")
                    nc.vector.tensor_mul(t2[:], raw[:], cos_sb[:, sl])
                    t1 = ttp.tile([128, 512], f32, tag="tt")
                    nc.vector.tensor_mul(t1[:], rp[:], sin_sb[:, sl])
                    if ti == 1:
                        nc.vector.tensor_add(kT[g_][:, sl], t1[:], t2[:])
                    else:
                        nc.gpsimd.tensor_tensor(
                            qp[g_][:, sl], t1[:], t2[:], mybir.AluOpType.add
                        )

                def qk_chunk(g, ti, c):
                    coff = ti * HPG * D + 128 * g
                    sl = slice(512 * c, 512 * (c + 1))
                    pp = pjp.tile([128, 512], f32, tag="pj", name=f"pj{g}_{ti}_{c}")
                    for e in range(EK):
                        nc.tensor.matmul(
                            pp[:],
                            wqk_sb[e][:, coff : coff + 128],
                            xt_sb[e][c],
                            start=(e == 0),
                            stop=(e == EK - 1),
                        )
                    raw = rawp.tile([128, 512], f32r, tag="raw")
                    nc.vector.tensor_copy(raw[:], pp[:])
                    rope_pend.append((g, ti, sl, raw))
                    if len(rope_pend) > 1:
                        rope_tail()

                def v_st(st):
                    vp_ps = vpp.tile([128, 256], f32, tag="vp", name=f"vps{st}")
                    c, sub = divmod(st, 4)
                    for e in range(EK):
                        nc.tensor.matmul(
                            vp_ps[:],
                            xt_sb[e][c][:, 128 * sub : 128 * (sub + 1)],
                            wv_sb[e][:],
                            start=(e == 0),
                            stop=(e == EK - 1),
                        )
                    for h in range(4):
                        nc.vector.tensor_copy(
                            v_c[:, st, 65 * h : 65 * h + 64],
                            vp_ps[:, 64 * h : 64 * h + 64],
                        )

                for c in range(NCH):
                    qk_chunk(0, 1, c)            # k pair 0
                    qk_chunk(1, 1, c)            # k pair 1
                    qk_chunk(0, 0, c)            # q pair 0
                    qk_chunk(1, 0, c)            # q pair 1
                    # v last: its DVE copies have no downstream PE
                    # dependency, so the rope chains (which gate the first
                    # scores) never queue behind them
                    for st in range(4 * c, 4 * c + 4):
                        v_st(st)
                rope_tail()
                rope_tail()

            # ============ attention: pair-loop, ScalarE-saturated ===========
            fill = []

            with (
                tc.tile_pool(name="sAB", bufs=1, space="PSUM") as sABp,
                tc.tile_pool(name="oT", bufs=1, space="PSUM") as oTp,
            ):

                def out_items(st, n, tag):
                    ssl = slice(128 * st, 128 * (st + 1))
                    nsl = slice(512 * n, 512 * (n + 1))
                    cell = {}

                    def mk_mm(g):
                        def it():
                            if g == 0:
                                cell["op"] = sABp.tile(
                                    [128, 512], f32, tag=tag,
                                    name=f"op{st}_{n}"
                                )
                            nc.tensor.matmul(
                                cell["op"][:],
                                att_o[g][:, ssl],
                                wo_sb[g][:, nsl],
                                start=(g == 0),
                                stop=(g == PAIRS - 1),
                            )
                        return it

                    def ev_it():
                        ev = evp.tile([128, 512], bf16, tag="ev")
                        nc.vector.tensor_copy(ev[:], cell["op"][:])
                        nc.sync.dma_start(out[ssl, nsl], ev[:])

                    return [mk_mm(g) for g in range(PAIRS)] + [ev_it]

                def norm_head(g, ch, hh, oT):
                    # one fast 65-partition copy frees the PSUM accumulator
                    # for the next block; the slow single-partition staging
                    # and broadcast run asynchronously off SBUF afterwards.
                    rows = slice(64 * hh, 64 * hh + 64)
                    csl = slice(SQ_CHUNK * ch, SQ_CHUNK * (ch + 1))
                    oA = oap.tile([65, SQ_CHUNK], f32, tag="oA")
                    nc.vector.tensor_copy(oA[:], oT[:])
                    dn = oap.tile([1, SQ_CHUNK], f32, tag="dn")
                    nc.vector.tensor_copy(dn[:], oA[64:65, :])
                    rbb = oap.tile([64, SQ_CHUNK], f32, tag="rbb")
                    nc.gpsimd.partition_broadcast(rbb[:], dn[:])
                    rbr = oap.tile([64, SQ_CHUNK], f32, tag="rbr")
                    nc.vector.reciprocal_approx_fast(rbr[:], rbb[:])
                    nc.vector.tensor_mul(att_o[g][rows, csl], oA[0:64, :], rbr[:])

                def attention_block(g, ch):
                    oTA = oTp.tile([65, SQ_CHUNK], f32, tag="oTA",
                                   name=f"oTA{g}_{ch}")
                    oTB = oTp.tile([65, SQ_CHUNK], f32, tag="oTB",
                                   name=f"oTB{g}_{ch}")
                    exps = []

                    def attnv(j):
                        eA, eB = exps[j]
                        first, last = j == 0, j == N_SK - 1
                        for n in range(2):
                            nsl = slice(512 * n, 512 * (n + 1))
                            nc.tensor.matmul(
                                oTA[:, nsl], v_c[:, j, 130 * g : 130 * g + 65],
                                eA[:, nsl], start=first, stop=last,
                            )
                            nc.tensor.matmul(
                                oTB[:, nsl],
                                v_c[:, j, 130 * g + 65 : 130 * g + 130],
                                eB[:, nsl], start=first, stop=last,
                            )

                    for sk in range(N_SK):
                        sksl = slice(128 * sk, 128 * (sk + 1))
                        sA = sABp.tile([128, SQ_CHUNK], f32, tag="sA",
                                       name=f"sA{g}_{ch}_{sk}")
                        sB = sABp.tile([128, SQ_CHUNK], f32, tag="sB",
                                       name=f"sB{g}_{ch}_{sk}")
                        for n in range(2):
                            gsl = slice(SQ_CHUNK * ch + 512 * n,
                                        SQ_CHUNK * ch + 512 * (n + 1))
                            nc.tensor.matmul(
                                sA[:, 512 * n : 512 * (n + 1)],
                                kT[g][0:64, sksl], qp[g][0:64, gsl],
                                start=True, stop=True,
                            )
                            nc.tensor.matmul(
                                sB[:, 512 * n : 512 * (n + 1)],
                                kT[g][64:128, sksl], qp[g][64:128, gsl],
                                start=True, stop=True,
                            )
                        eA = expp.tile([128, SQ_CHUNK], bf16, tag="eA")
                        nc.scalar.activation(eA[:], sA[:], AF.Exp, scale=0.125)
                        eB = expp.tile([128, SQ_CHUNK], bf16, tag="eB")
                        nc.scalar.activation(eB[:], sB[:], AF.Exp, scale=0.125)
                        exps.append((eA, eB))
                        if sk > 0:
                            attnv(sk - 1)
                        if fill:
                            fill.pop(0)()
                    attnv(N_SK - 1)
                    norm_head(g, ch, 0, oTA)
                    norm_head(g, ch, 1, oTB)

                for ch in range(N_CH):
                    for g in range(PAIRS):
                        attention_block(g, ch)

                # output projection: one dense burst after attention (dense
                # matmuls keep the PE clock warm; ScalarE is finished)
                for st in range(S // 128):
                    for n in range(2):
                        for it in out_items(st, n, "sA" if (st + n) % 2 == 0 else "sB"):
                            it()

    nc.compile()
    return nc


def _get_program():
    if "nc" not in _BUILT:
        _BUILT["nc"] = _build_program()
    return _BUILT["nc"]


def _host_inputs(x, W_qkv, W_out):
    """Build the 8 per-core input maps."""
    import ml_dtypes

    f = np.float32
    bf = ml_dtypes.bfloat16
    x = np.asarray(x, dtype=f)
    W_qkv = np.asarray(W_qkv, dtype=f)
    W_out = np.asarray(W_out, dtype=f)

    inv_freq = 1.0 / (ROPE_THETA ** (np.arange(0, D, 2, dtype=np.float64) / D))
    p = np.arange(128)
    freq_row = inv_freq[(p % D) // 2]  # [128]
    ang = freq_row[:, None] * np.arange(S, dtype=np.float64)[None, :]  # [128, S]
    cos_t = np.cos(ang).astype(f)
    sign = np.where(p % 2 == 0, -1.0, 1.0)[:, None]
    sin_t = (np.sin(ang) * sign).astype(f)

    msw = np.zeros((128, 128), dtype=f)
    msw[p, p ^ 1] = 1.0

    maps = []
    for core in range(N_CORES):
        b, hg = divmod(core, HG)
        hs = [HPG * hg + i for i in range(HPG)]
        w_qk = np.concatenate(
            [W_qkv[:, h * D : (h + 1) * D] for h in hs]
            + [W_qkv[:, ATT + h * D : ATT + (h + 1) * D] for h in hs],
            axis=1,
        )
        w_v = np.concatenate(
            [W_qkv[:, 2 * ATT + h * D : 2 * ATT + (h + 1) * D] for h in hs], axis=1
        )
        w_o = np.concatenate([W_out[h * D : (h + 1) * D, :] for h in hs], axis=0)
        maps.append(
            {
                "xT": np.ascontiguousarray(x[b].T).astype(bf),
                "w_qk": np.ascontiguousarray(w_qk).astype(bf),
                "w_v": np.ascontiguousarray(w_v).astype(bf),
                "w_o": np.ascontiguousarray(w_o).astype(bf),
                "cos_t": cos_t,
                "sin_t": sin_t,
                "mswap": msw,
            }
        )
    return maps


def kernel(x, W_qkv, W_out):
    from concourse.bass_utils import run_bass_kernel_spmd

    nc = _get_program()
    maps = _host_inputs(x, W_qkv, W_out)
    res = run_bass_kernel_spmd(nc, maps, core_ids=list(range(N_CORES)))
    out = np.zeros((B, S, E), dtype=np.float32)
    for core in range(N_CORES):
        b = core // HG
        out[b] += np.asarray(res.results[core]["out"], dtype=np.float32)
    return out
